# revision 25
# baseline (speedup 1.0000x reference)
"""Trainium2 Bass kernel for nn_LogicDense (difflogic dense layer).

Math (reference):
    w      = softmax(weight, axis=-1)            # [out_dim, 16]
    coeffs = w @ GATE_COEFFS                     # [out_dim, 4] = (c0, ca, cb, cab)
    a      = x[:, indices[0]]                    # [batch, out_dim]
    b      = x[:, indices[1]]
    out    = c0 + ca*a + cb*b + cab*a*b          # [batch, out_dim]

Strategy (8 NeuronCores, tensor-parallel over out_dim):
    - Host transposes x -> x_t [in_dim, batch] (fp16, replicated to all
      cores).
    - Core c owns output rows j in [2048*c, 2048*(c+1)).
    - Per 128-row chunk: one GPSIMD dma_gather pulls the 256 rows
      x_t[idx0[chunk]] ++ x_t[idx1[chunk]] from HBM into SBUF (row i of the
      index list lands on partition i%128, slot i//128; full batch on the
      free dim).
    - Per-partition coeff scalars then give a 4-instruction combine:
         t = cab*b + ca      (ACT: Identity activation, scale/bias APs)
         h = cb*b + c0       (ACT)
         o = t * a           (DVE tensor_mul, in place over t)
         o = o + h           (DVE tensor_add)
    - Softmax+gate-coeff collapse is computed on device (ACT exp + DVE
      reduces, fp32) from the raw weight shard.
    - Core output is [2048, 4096] fp16 (out_dim-major); host concatenates
      the 8 shards, transposes back to [batch, out_dim], upcasts to fp32.
    - 3-deep buffer rotation so gather DMA, ACT, DVE and store DMA all
      overlap; per-buffer rotating DMA semaphores.
"""

import os
import sys

import numpy as np

sys.path.insert(0, "/opt/trn_rl_repo")

BATCH = 4096
IN_DIM = 8192
OUT_DIM = 16384
N_CORES = 8
J_SHARD = OUT_DIM // N_CORES        # 2048 output rows per core
CHUNK = 128                         # output rows per pipeline iteration
N_CHUNKS = J_SHARD // CHUNK         # 16
NB = 4                              # pipeline buffer sets
DVE_PRE = 14                        # DVE preamble (coeff) instruction count

GATE_COEFFS = np.array([
    [0, 0, 0, 0], [0, 0, 0, 1], [0, 1, 0, -1], [0, 1, 0, 0],
    [0, 0, 1, -1], [0, 0, 1, 0], [0, 1, 1, -2], [0, 1, 1, -1],
    [1, -1, -1, 1], [1, -1, -1, 2], [1, 0, -1, 0], [1, 0, -1, 1],
    [1, -1, 0, 0], [1, -1, 0, 1], [1, 0, 0, -1], [1, 0, 0, 0],
], dtype=np.float32)                # [16 gates, 4 bilinear coeffs]

_CACHE = {}
LAST_RESULT = None  # BassKernelResults of the most recent run (for profiling)


def _wrap_idx16(idx_pair):
    """Wrap the per-core [2, J_SHARD] index array into dma_gather's index
    layout. Per 128-row chunk c the kernel issues ONE gather of 256 indices
    (idx0[chunk] ++ idx1[chunk]); index i of that list lives at
    [i%16, 16*c + i//16], and the 16-partition block is replicated across
    all 8 groups of 16 partitions (the Q7 tx/rx cpus read the indices from
    different partition groups)."""
    cols = []
    for c in range(N_CHUNKS):
        merged = np.concatenate([idx_pair[0, c * CHUNK:(c + 1) * CHUNK],
                                 idx_pair[1, c * CHUNK:(c + 1) * CHUNK]])
        cols.append(merged.astype(np.int16).reshape(16, 16).T)  # [16, 16]
    blk = np.concatenate(cols, axis=1)                # [16, 16*N_CHUNKS]
    return np.ascontiguousarray(np.tile(blk, (8, 1)))  # [128, 256]


def _build_program():
    import concourse.bacc as bacc
    import concourse.mybir as mybir
    from concourse.library_config import mlp
    from contextlib import ExitStack

    dt = mybir.dt
    AF = mybir.ActivationFunctionType

    nc = bacc.Bacc("TRN2", target_bir_lowering=False, debug=False)

    xt = nc.dram_tensor("xt", [IN_DIM, BATCH], dt.float16, kind="ExternalInput")
    idx = nc.dram_tensor("idx", [128, 2 * (J_SHARD // 16)], dt.int16,
                         kind="ExternalInput")
    wgt = nc.dram_tensor("wgt", [128, N_CHUNKS * 16], dt.float32,
                         kind="ExternalInput")
    gcr = nc.dram_tensor("gcr", [128, 4 * N_CHUNKS * 16], dt.float32,
                         kind="ExternalInput")
    out = nc.dram_tensor("out", [J_SHARD, BATCH], dt.float16,
                         kind="ExternalOutput")

    W16 = N_CHUNKS * 16  # 256: free size of the wrapped weight / exp tiles

    with ExitStack() as ctx:
        sb = lambda name, shape, dty: ctx.enter_context(
            nc.sbuf_tensor(name, shape, dty))
        sb_idx = sb("sb_idx", [128, 2 * (J_SHARD // 16)], dt.int16)
        sb_w = sb("sb_w", [128, W16], dt.float32)
        sb_gc = sb("sb_gc", [128, 4 * W16], dt.float32)
        sb_e = sb("sb_e", [128, W16], dt.float32)
        sb_scr = sb("sb_scr", [128, W16], dt.float32)
        sb_s = sb("sb_s", [128, N_CHUNKS], dt.float32)
        sb_r = sb("sb_r", [128, N_CHUNKS], dt.float32)
        # coeff tile: [:, 16*k + c] = coeff k (0=c0,1=ca,2=cb,3=cab), chunk c
        sb_cc = sb("sb_cc", [128, 4 * N_CHUNKS], dt.float32)
        # gather dst: slot 0 = a rows, slot 1 = b rows
        ab_bufs = [sb(f"ab{k}", [128, 2, BATCH], dt.float16) for k in range(NB)]
        t_bufs = [sb(f"t{k}", [128, BATCH], dt.float16) for k in range(NB)]
        h_bufs = [sb(f"h{k}", [128, BATCH], dt.float16) for k in range(NB)]

        # The t-op (t = cab*b + ca) alternates between ACT (even iters) and
        # DVE tensor_scalar (odd iters) to balance the two engines under the
        # gather-DMA roofline. Both streams are software-pipelined one slot
        # ahead (t(i+1) is emitted before h(i) / between mul(i) and add(i))
        # so every same-engine and cross-engine dependency has >=1
        # instruction of separation and sem waits are pre-satisfied.
        t_on_act = [i % 2 == 0 for i in range(N_CHUNKS)]

        ops_act = []  # ACT stream after the exp op: ('t', j) / ('h', i)
        if t_on_act[0]:
            ops_act.append(('t', 0))
        for i in range(N_CHUNKS):
            if i + 1 < N_CHUNKS and t_on_act[i + 1]:
                ops_act.append(('t', i + 1))
            ops_act.append(('h', i))
        act_val = {op: n + 1 for n, op in enumerate(ops_act)}

        ops_dve = []  # DVE stream after the coeff preamble
        if not t_on_act[0]:
            ops_dve.append(('ts', 0))
        for i in range(N_CHUNKS):
            ops_dve.append(('mul', i))
            if i + 1 < N_CHUNKS and not t_on_act[i + 1]:
                ops_dve.append(('ts', i + 1))
            ops_dve.append(('add', i))
        dve_val = {op: DVE_PRE + n + 1 for n, op in enumerate(ops_dve)}

        with (
            nc.Block() as block,
            nc.semaphore("s_pi") as s_pi,
            nc.semaphore("s_pw") as s_pw,
            nc.semaphore("s_pg") as s_pg,
            nc.semaphore("s_exp") as s_exp,
            nc.semaphore("s_g0") as s_g0,
            nc.semaphore("s_g1") as s_g1,
            nc.semaphore("s_g2") as s_g2,
            nc.semaphore("s_g3") as s_g3,
            nc.semaphore("s_st0") as s_st0,
            nc.semaphore("s_st1") as s_st1,
            nc.semaphore("s_st2") as s_st2,
            nc.semaphore("s_st3") as s_st3,
            nc.semaphore("s_act") as s_act,
            nc.semaphore("s_dve") as s_dve,
        ):
            s_g = [s_g0, s_g1, s_g2, s_g3]
            s_st = [s_st0, s_st1, s_st2, s_st3]

            def cseg(k, i):  # per-partition scalar AP: coeff k, chunk i
                return sb_cc[:, 16 * k + i : 16 * k + i + 1]

            @block.sync
            def _(sync):
                sync.dma_start(sb_idx[:, :], idx[:, :]).then_inc(s_pi, 16)
                sync.dma_start(sb_w[:, :], wgt[:, :]).then_inc(s_pw, 16)
                sync.dma_start(sb_gc[:, :], gcr[:, :]).then_inc(s_pg, 16)
                for i in range(N_CHUNKS):
                    k = i % NB
                    sync.wait_ge(s_dve, dve_val[('add', i)])
                    if i >= NB:
                        sync.wait_ge(s_st[k], 16 * (i // NB))
                    sync.dma_start(out[i * CHUNK:(i + 1) * CHUNK, :],
                                   t_bufs[k][:, :]).then_inc(s_st[k], 16)
                for k in range(NB):
                    n_st = (N_CHUNKS - 1 - k) // NB + 1
                    sync.wait_ge(s_st[k], 16 * n_st)

            @block.gpsimd
            def _(gp):
                gp.load_library(mlp)
                nreg = gp.alloc_register("nidx")
                gp.reg_mov(nreg, 2 * CHUNK)
                gp.wait_ge(s_pi, 16)  # idx tile loaded
                for i in range(N_CHUNKS):
                    k = i % NB
                    if i >= NB:
                        # ab[k] free once iter i-NB's b/a readers are done:
                        # DVE mul (a; ts read b before it) + ACT ops (b).
                        gp.wait_ge(s_dve, dve_val[('mul', i - NB)])
                        gp.wait_ge(s_act, act_val[('h', i - NB)])
                        gp.wait_ge(s_g[k], 16 * (i // NB))
                    gp.dma_gather(
                        ab_bufs[k].ap(), xt.ap(),
                        sb_idx[:, 16 * i:16 * i + 16], 2 * CHUNK, nreg, BATCH,
                    ).then_inc(s_g[k], 16)

            @block.scalar
            def _(sc):
                sc.wait_ge(s_pw, 16)
                sc.activation(sb_e[:, :], sb_w[:, :], AF.Exp).then_inc(s_exp, 1)
                sc.wait_ge(s_dve, DVE_PRE)  # coeff tile ready
                for kind, i in ops_act:
                    k = i % NB
                    sc.wait_ge(s_g[k], 16 * (i // NB + 1))
                    if kind == 't':
                        # t[k] free once store of i-NB completed
                        if i >= NB:
                            sc.wait_ge(s_st[k], 16 * (i // NB))
                        sc.activation(t_bufs[k][:, :], ab_bufs[k][:, 1, :],
                                      AF.Identity,
                                      bias=cseg(1, i), scale=cseg(3, i),
                                      ).then_inc(s_act, 1)
                    else:
                        # h[k] free once DVE add of i-NB completed
                        if i >= NB:
                            sc.wait_ge(s_dve, dve_val[('add', i - NB)])
                        sc.activation(h_bufs[k][:, :], ab_bufs[k][:, 1, :],
                                      AF.Identity,
                                      bias=cseg(0, i), scale=cseg(2, i),
                                      ).then_inc(s_act, 1)

            @block.vector
            def _(v):
                # The DVE pipeline is deep: every same-engine RAW below is
                # chained through s_dve (each op incs by 1, dependents wait).
                X = mybir.AxisListType.X
                n = 0

                def step(ins):
                    nonlocal n
                    n += 1
                    ins.then_inc(s_dve, 1)

                v.wait_ge(s_exp, 1)
                v.wait_ge(s_pg, 16)  # gc tile loaded
                e3 = sb_e[:, :].rearrange("p (c g) -> p c g", g=16)
                step(v.reduce_sum(sb_s[:, :], e3, axis=X))
                v.wait_ge(s_dve, n)
                step(v.reciprocal(sb_r[:, :], sb_s[:, :]))
                for kk in range(4):
                    if kk > 0:
                        v.wait_ge(s_dve, n)  # scr free (prior reduce read it)
                    step(v.tensor_mul(sb_scr[:, :], sb_e[:, :],
                                      sb_gc[:, kk * W16:(kk + 1) * W16]))
                    v.wait_ge(s_dve, n)
                    step(v.reduce_sum(
                        sb_cc[:, 16 * kk:16 * (kk + 1)],
                        sb_scr[:, :].rearrange("p (c g) -> p c g", g=16),
                        axis=X))
                v.wait_ge(s_dve, n)  # all cc segments + r landed
                for kk in range(4):
                    step(v.tensor_mul(sb_cc[:, 16 * kk:16 * (kk + 1)],
                                      sb_cc[:, 16 * kk:16 * (kk + 1)],
                                      sb_r[:, :]))
                assert n == DVE_PRE
                v.wait_ge(s_dve, DVE_PRE)  # cc normalize landed
                MU, AD = mybir.AluOpType.mult, mybir.AluOpType.add
                for kind, i in ops_dve:
                    k = i % NB
                    if kind == 'ts':
                        # t = (b * cab) + ca  (fp16 tensor_scalar, 4x mode)
                        v.wait_ge(s_g[k], 16 * (i // NB + 1))
                        if i >= NB:
                            # t[k] free once store of iter i-NB completed
                            v.wait_ge(s_st[k], 16 * (i // NB))
                        v.tensor_scalar(t_bufs[k][:, :], ab_bufs[k][:, 1, :],
                                        cseg(3, i), cseg(1, i), MU, AD,
                                        ).then_inc(s_dve, 1)
                    elif kind == 'mul':
                        v.wait_ge(s_g[k], 16 * (i // NB + 1))
                        if t_on_act[i]:
                            if i >= NB:
                                v.wait_ge(s_st[k], 16 * (i // NB))
                            v.wait_ge(s_act, act_val[('t', i)])
                        else:
                            v.wait_ge(s_dve, dve_val[('ts', i)])
                        v.tensor_mul(t_bufs[k][:, :], t_bufs[k][:, :],
                                     ab_bufs[k][:, 0, :]).then_inc(s_dve, 1)
                    else:  # add
                        v.wait_ge(s_act, act_val[('h', i)])
                        v.wait_ge(s_dve, dve_val[('mul', i)])
                        v.tensor_add(t_bufs[k][:, :], t_bufs[k][:, :],
                                     h_bufs[k][:, :]).then_inc(s_dve, 1)

    nc.compile()
    return nc


def _get_program():
    if "nc" not in _CACHE:
        _CACHE["nc"] = _build_program()
    return _CACHE["nc"]


def kernel(x, weight, indices):
    global LAST_RESULT
    from concourse.bass_utils import run_bass_kernel_spmd

    x = np.asarray(x, dtype=np.float32)
    weight = np.asarray(weight, dtype=np.float32)
    indices = np.asarray(indices)

    nc = _get_program()

    xt = np.ascontiguousarray(x.T.astype(np.float16))    # [in_dim, batch]

    # gc replicate: [p, kk*256 + 16*c + g] = GATE_COEFFS[g, kk]
    gc_rep = np.broadcast_to(
        GATE_COEFFS.T.reshape(4, 1, 16),                 # [kk, 1, g]
        (4, N_CHUNKS, 16)).reshape(1, -1)
    gc_rep = np.ascontiguousarray(
        np.broadcast_to(gc_rep, (128, 4 * N_CHUNKS * 16)).astype(np.float32))

    in_maps = []
    for c in range(N_CORES):
        j0 = c * J_SHARD
        idx_c = _wrap_idx16(indices[:, j0:j0 + J_SHARD])
        wsh = weight[j0:j0 + J_SHARD]                    # [2048, 16]
        w_wrapped = np.ascontiguousarray(
            wsh.reshape(N_CHUNKS, 128, 16).transpose(1, 0, 2)
            .reshape(128, N_CHUNKS * 16))
        in_maps.append({
            "xt": xt,
            "idx": idx_c,
            "wgt": w_wrapped,
            "gcr": gc_rep,
        })

    trace = bool(os.environ.get("KERNEL_TRACE"))
    res = run_bass_kernel_spmd(nc, in_maps, core_ids=list(range(N_CORES)),
                               trace=trace)
    LAST_RESULT = res

    shards = [res.results[c]["out"] for c in range(N_CORES)]
    full = np.concatenate(shards, axis=0)                # [out_dim, batch]
    return np.ascontiguousarray(full.T.astype(np.float32))


# revision 28
# speedup vs baseline: 1.2124x; 1.2124x over previous
"""Trainium2 Bass kernel for nn_LogicDense (difflogic dense layer).

Math (reference):
    w      = softmax(weight, axis=-1)            # [out_dim, 16]
    coeffs = w @ GATE_COEFFS                     # [out_dim, 4] = (c0, ca, cb, cab)
    a      = x[:, indices[0]]                    # [batch, out_dim]
    b      = x[:, indices[1]]
    out    = c0 + ca*a + cb*b + cab*a*b          # [batch, out_dim]

Strategy (8 NeuronCores, tensor-parallel over out_dim):
    - Host transposes x -> x_t [in_dim, batch] (fp16, replicated to all
      cores).
    - Core c owns output rows j in [2048*c, 2048*(c+1)).
    - Per 128-row chunk: one GPSIMD dma_gather pulls the 256 rows
      x_t[idx0[chunk]] ++ x_t[idx1[chunk]] from HBM into SBUF (row i of the
      index list lands on partition i%128, slot i//128; full batch on the
      free dim).
    - Per-partition coeff scalars then give a 4-instruction combine:
         t = cab*b + ca      (ACT: Identity activation, scale/bias APs)
         h = cb*b + c0       (ACT)
         o = t * a           (DVE tensor_mul, in place over t)
         o = o + h           (DVE tensor_add)
    - Softmax+gate-coeff collapse is computed on device (ACT exp + DVE
      reduces, fp32) from the raw weight shard.
    - Core output is [2048, 4096] fp16 (out_dim-major); host concatenates
      the 8 shards, transposes back to [batch, out_dim], upcasts to fp32.
    - 3-deep buffer rotation so gather DMA, ACT, DVE and store DMA all
      overlap; per-buffer rotating DMA semaphores.
"""

import os
import sys

import numpy as np

sys.path.insert(0, "/opt/trn_rl_repo")

BATCH = 4096
IN_DIM = 8192
OUT_DIM = 16384
N_CORES = 8
J_SHARD = OUT_DIM // N_CORES        # 2048 output rows per core
CHUNK = 128                         # output rows per pipeline iteration
N_CHUNKS = J_SHARD // CHUNK         # 16
NB = 4                              # pipeline buffer sets
DVE_PRE = 14                        # DVE preamble (coeff) instruction count

GATE_COEFFS = np.array([
    [0, 0, 0, 0], [0, 0, 0, 1], [0, 1, 0, -1], [0, 1, 0, 0],
    [0, 0, 1, -1], [0, 0, 1, 0], [0, 1, 1, -2], [0, 1, 1, -1],
    [1, -1, -1, 1], [1, -1, -1, 2], [1, 0, -1, 0], [1, 0, -1, 1],
    [1, -1, 0, 0], [1, -1, 0, 1], [1, 0, 0, -1], [1, 0, 0, 0],
], dtype=np.float32)                # [16 gates, 4 bilinear coeffs]

_CACHE = {}
LAST_RESULT = None  # BassKernelResults of the most recent run (for profiling)


def _wrap_idx16(idx_pair):
    """Wrap the per-core [2, J_SHARD] index array into dma_gather's index
    layout. Per 128-row chunk c the kernel issues ONE gather of 256 indices
    (idx0[chunk] ++ idx1[chunk]); index i of that list lives at
    [i%16, 16*c + i//16], and the 16-partition block is replicated across
    all 8 groups of 16 partitions (the Q7 tx/rx cpus read the indices from
    different partition groups)."""
    cols = []
    for c in range(N_CHUNKS):
        merged = np.concatenate([idx_pair[0, c * CHUNK:(c + 1) * CHUNK],
                                 idx_pair[1, c * CHUNK:(c + 1) * CHUNK]])
        cols.append(merged.astype(np.int16).reshape(16, 16).T)  # [16, 16]
    blk = np.concatenate(cols, axis=1)                # [16, 16*N_CHUNKS]
    return np.ascontiguousarray(np.tile(blk, (8, 1)))  # [128, 256]


def _build_program():
    import concourse.bacc as bacc
    import concourse.mybir as mybir
    from concourse.library_config import mlp
    from contextlib import ExitStack

    dt = mybir.dt
    AF = mybir.ActivationFunctionType

    nc = bacc.Bacc("TRN2", target_bir_lowering=False, debug=False)

    xt = nc.dram_tensor("xt", [IN_DIM, BATCH], dt.float16, kind="ExternalInput")
    idx = nc.dram_tensor("idx", [128, 2 * (J_SHARD // 16)], dt.int16,
                         kind="ExternalInput")
    wgt = nc.dram_tensor("wgt", [128, N_CHUNKS * 16], dt.float32,
                         kind="ExternalInput")
    gcr = nc.dram_tensor("gcr", [128, 4 * N_CHUNKS * 16], dt.float32,
                         kind="ExternalInput")
    out = nc.dram_tensor("out", [J_SHARD, BATCH], dt.float16,
                         kind="ExternalOutput")

    W16 = N_CHUNKS * 16  # 256: free size of the wrapped weight / exp tiles

    with ExitStack() as ctx:
        sb = lambda name, shape, dty: ctx.enter_context(
            nc.sbuf_tensor(name, shape, dty))
        sb_idx = sb("sb_idx", [128, 2 * (J_SHARD // 16)], dt.int16)
        sb_w = sb("sb_w", [128, W16], dt.float32)
        sb_gc = sb("sb_gc", [128, 4 * W16], dt.float32)
        sb_e = sb("sb_e", [128, W16], dt.float32)
        sb_scr = sb("sb_scr", [128, W16], dt.float32)
        sb_s = sb("sb_s", [128, N_CHUNKS], dt.float32)
        sb_r = sb("sb_r", [128, N_CHUNKS], dt.float32)
        # coeff tile: [:, 16*k + c] = coeff k (0=c0,1=ca,2=cb,3=cab), chunk c
        sb_cc = sb("sb_cc", [128, 4 * N_CHUNKS], dt.float32)
        # gather dst: slot 0 = a rows, slot 1 = b rows
        ab_bufs = [sb(f"ab{k}", [128, 2, BATCH], dt.float16) for k in range(NB)]
        t_bufs = [sb(f"t{k}", [128, BATCH], dt.float16) for k in range(NB)]
        h_bufs = [sb(f"h{k}", [128, BATCH], dt.float16) for k in range(NB)]

        # ACT computes h = cb*b + c0 (one op/iter); DVE computes
        # t = cab*b + ca (tensor_scalar, 4x), g = t*a, o = g + h.
        # The DVE stream is software-pipelined backward one slot —
        # [ts(i), add(i-1), mul(i)] — so every same-engine RAW has one
        # instruction of separation (sem waits pre-satisfied) and nothing
        # gates on a future gather.
        ops_act = [('h', i) for i in range(N_CHUNKS)]
        act_val = {op: n + 1 for n, op in enumerate(ops_act)}

        ops_dve = []  # DVE stream after the coeff preamble
        for i in range(N_CHUNKS):
            ops_dve.append(('ts', i))
            if i > 0:
                ops_dve.append(('add', i - 1))
            ops_dve.append(('mul', i))
        ops_dve.append(('add', N_CHUNKS - 1))
        dve_val = {op: DVE_PRE + n + 1 for n, op in enumerate(ops_dve)}

        with (
            nc.Block() as block,
            nc.semaphore("s_pi") as s_pi,
            nc.semaphore("s_pw") as s_pw,
            nc.semaphore("s_pg") as s_pg,
            nc.semaphore("s_exp") as s_exp,
            nc.semaphore("s_g0") as s_g0,
            nc.semaphore("s_g1") as s_g1,
            nc.semaphore("s_g2") as s_g2,
            nc.semaphore("s_g3") as s_g3,
            nc.semaphore("s_st0") as s_st0,
            nc.semaphore("s_st1") as s_st1,
            nc.semaphore("s_st2") as s_st2,
            nc.semaphore("s_st3") as s_st3,
            nc.semaphore("s_act") as s_act,
            nc.semaphore("s_dve") as s_dve,
        ):
            s_g = [s_g0, s_g1, s_g2, s_g3]
            s_st = [s_st0, s_st1, s_st2, s_st3]

            def cseg(k, i):  # per-partition scalar AP: coeff k, chunk i
                return sb_cc[:, 16 * k + i : 16 * k + i + 1]

            @block.sync
            def _(sync):
                sync.dma_start(sb_idx[:, :], idx[:, :]).then_inc(s_pi, 16)
                sync.dma_start(sb_w[:, :], wgt[:, :]).then_inc(s_pw, 16)
                sync.dma_start(sb_gc[:, :], gcr[:, :]).then_inc(s_pg, 16)
                for i in range(N_CHUNKS):
                    k = i % NB
                    sync.wait_ge(s_dve, dve_val[('add', i)])
                    if i >= NB:
                        sync.wait_ge(s_st[k], 16 * (i // NB))
                    sync.dma_start(out[i * CHUNK:(i + 1) * CHUNK, :],
                                   t_bufs[k][:, :]).then_inc(s_st[k], 16)
                for k in range(NB):
                    n_st = (N_CHUNKS - 1 - k) // NB + 1
                    sync.wait_ge(s_st[k], 16 * n_st)

            @block.gpsimd
            def _(gp):
                gp.load_library(mlp)
                nreg = gp.alloc_register("nidx")
                gp.reg_mov(nreg, 2 * CHUNK)
                gp.wait_ge(s_pi, 16)  # idx tile loaded
                for i in range(N_CHUNKS):
                    k = i % NB
                    if i >= NB:
                        # ab[k] free once iter i-NB's b/a readers are done:
                        # DVE mul (a; ts read b before it) + ACT ops (b).
                        gp.wait_ge(s_dve, dve_val[('mul', i - NB)])
                        gp.wait_ge(s_act, act_val[('h', i - NB)])
                        gp.wait_ge(s_g[k], 16 * (i // NB))
                    gp.dma_gather(
                        ab_bufs[k].ap(), xt.ap(),
                        sb_idx[:, 16 * i:16 * i + 16], 2 * CHUNK, nreg, BATCH,
                    ).then_inc(s_g[k], 16)

            @block.scalar
            def _(sc):
                sc.wait_ge(s_pw, 16)
                sc.activation(sb_e[:, :], sb_w[:, :], AF.Exp).then_inc(s_exp, 1)
                sc.wait_ge(s_dve, DVE_PRE)  # coeff tile ready
                for kind, i in ops_act:
                    k = i % NB
                    sc.wait_ge(s_g[k], 16 * (i // NB + 1))
                    # h[k] free once DVE add of i-NB completed
                    if i >= NB:
                        sc.wait_ge(s_dve, dve_val[('add', i - NB)])
                    sc.activation(h_bufs[k][:, :], ab_bufs[k][:, 1, :],
                                  AF.Identity,
                                  bias=cseg(0, i), scale=cseg(2, i),
                                  ).then_inc(s_act, 1)

            @block.vector
            def _(v):
                # The DVE pipeline is deep: every same-engine RAW below is
                # chained through s_dve (each op incs by 1, dependents wait).
                X = mybir.AxisListType.X
                n = 0

                def step(ins):
                    nonlocal n
                    n += 1
                    ins.then_inc(s_dve, 1)

                v.wait_ge(s_exp, 1)
                v.wait_ge(s_pg, 16)  # gc tile loaded
                e3 = sb_e[:, :].rearrange("p (c g) -> p c g", g=16)
                step(v.reduce_sum(sb_s[:, :], e3, axis=X))
                v.wait_ge(s_dve, n)
                step(v.reciprocal(sb_r[:, :], sb_s[:, :]))
                for kk in range(4):
                    if kk > 0:
                        v.wait_ge(s_dve, n)  # scr free (prior reduce read it)
                    step(v.tensor_mul(sb_scr[:, :], sb_e[:, :],
                                      sb_gc[:, kk * W16:(kk + 1) * W16]))
                    v.wait_ge(s_dve, n)
                    step(v.reduce_sum(
                        sb_cc[:, 16 * kk:16 * (kk + 1)],
                        sb_scr[:, :].rearrange("p (c g) -> p c g", g=16),
                        axis=X))
                v.wait_ge(s_dve, n)  # all cc segments + r landed
                for kk in range(4):
                    step(v.tensor_mul(sb_cc[:, 16 * kk:16 * (kk + 1)],
                                      sb_cc[:, 16 * kk:16 * (kk + 1)],
                                      sb_r[:, :]))
                assert n == DVE_PRE
                v.wait_ge(s_dve, DVE_PRE)  # cc normalize landed
                MU, AD = mybir.AluOpType.mult, mybir.AluOpType.add
                for kind, i in ops_dve:
                    k = i % NB
                    if kind == 'ts':
                        # t = (b * cab) + ca  (fp16 tensor_scalar, 4x mode)
                        v.wait_ge(s_g[k], 16 * (i // NB + 1))
                        if i >= NB:
                            # t[k] free once store of iter i-NB completed
                            v.wait_ge(s_st[k], 16 * (i // NB))
                        v.tensor_scalar(t_bufs[k][:, :], ab_bufs[k][:, 1, :],
                                        cseg(3, i), cseg(1, i), MU, AD,
                                        ).then_inc(s_dve, 1)
                    elif kind == 'mul':
                        v.wait_ge(s_dve, dve_val[('ts', i)])
                        v.tensor_mul(t_bufs[k][:, :], t_bufs[k][:, :],
                                     ab_bufs[k][:, 0, :]).then_inc(s_dve, 1)
                    else:  # add
                        v.wait_ge(s_act, act_val[('h', i)])
                        v.wait_ge(s_dve, dve_val[('mul', i)])
                        v.tensor_add(t_bufs[k][:, :], t_bufs[k][:, :],
                                     h_bufs[k][:, :]).then_inc(s_dve, 1)

    nc.compile()
    return nc


def _get_program():
    if "nc" not in _CACHE:
        _CACHE["nc"] = _build_program()
    return _CACHE["nc"]


def kernel(x, weight, indices):
    global LAST_RESULT
    from concourse.bass_utils import run_bass_kernel_spmd

    x = np.asarray(x, dtype=np.float32)
    weight = np.asarray(weight, dtype=np.float32)
    indices = np.asarray(indices)

    nc = _get_program()

    xt = np.ascontiguousarray(x.T.astype(np.float16))    # [in_dim, batch]

    # gc replicate: [p, kk*256 + 16*c + g] = GATE_COEFFS[g, kk]
    gc_rep = np.broadcast_to(
        GATE_COEFFS.T.reshape(4, 1, 16),                 # [kk, 1, g]
        (4, N_CHUNKS, 16)).reshape(1, -1)
    gc_rep = np.ascontiguousarray(
        np.broadcast_to(gc_rep, (128, 4 * N_CHUNKS * 16)).astype(np.float32))

    in_maps = []
    for c in range(N_CORES):
        j0 = c * J_SHARD
        idx_c = _wrap_idx16(indices[:, j0:j0 + J_SHARD])
        wsh = weight[j0:j0 + J_SHARD]                    # [2048, 16]
        w_wrapped = np.ascontiguousarray(
            wsh.reshape(N_CHUNKS, 128, 16).transpose(1, 0, 2)
            .reshape(128, N_CHUNKS * 16))
        in_maps.append({
            "xt": xt,
            "idx": idx_c,
            "wgt": w_wrapped,
            "gcr": gc_rep,
        })

    trace = bool(os.environ.get("KERNEL_TRACE"))
    res = run_bass_kernel_spmd(nc, in_maps, core_ids=list(range(N_CORES)),
                               trace=trace)
    LAST_RESULT = res

    shards = [res.results[c]["out"] for c in range(N_CORES)]
    full = np.concatenate(shards, axis=0)                # [out_dim, batch]
    return np.ascontiguousarray(full.T.astype(np.float32))
